# revision 1
# baseline (speedup 1.0000x reference)
"""Two-layer GAT on 8 TRN2 NeuronCores (Bass/Tile) — v2.

Strategy (self-contained; structure derived from edge_index at build time):
- Nodes/dst-segments sharded 8 ways (12500 own-dst nodes per core).
- Per core, its edges are sorted by dst and packed into 128-edge chunks that
  cover <=8 whole dst segments; 16 chunks = one 128-node "region" of a padded
  node space. All per-core structure differences are carried via input
  tensors; the instruction stream is identical (SPMD, one NEFF).
- Per layer: a bf16 feature table ([rows, 65] = [xs | a_s]) lives in DRAM;
  each core gathers its edges' source rows with ONE batched indirect DMA per
  4 regions (8192 rows/instruction — amortizes the ~1us SWDGE fixed cost).
- Max-free segment softmax: w = exp(leaky(a_s+a_d)); per chunk one PE matmul
  (lhsT = gathered rows [128e,65] bf16, rhs = w-valued one-hot [128e,8] bf16)
  accumulates [sum w*xs | sum w] into a transposed psum region [65,128]; the
  ones-column (row 64 overwritten with 1) gives denominators for free.
- Layer-1 finish projects table2 rows (h @ [W_src2|W_src2@att2]) immediately
  and writes them to a local DRAM block; quarters are AllGathered (bf16,
  Shared scratchpad) directly into the layer-2 gather table — no separate
  table-build phase for layer 2.
"""
import numpy as np
import ml_dtypes

BF16 = ml_dtypes.bfloat16

N = 100000
E = 1600000
D = 128
H = 64
NC = 8
NOWN = N // NC
WIN = 8      # dst nodes per chunk window
CHE = 128    # edges per chunk
RPC = 16     # chunks per region
NEG = 0.2
B = 4        # regions per gather batch


def _prep(edge_index):
    src_g = edge_index[0].astype(np.int64)
    dst_g = edge_index[1].astype(np.int64)
    cores = []
    for c in range(NC):
        lo, hi = c * NOWN, (c + 1) * NOWN
        sel = np.where((dst_g >= lo) & (dst_g < hi))[0]
        order = np.argsort(dst_g[sel], kind="stable")
        es = src_g[sel][order]
        ed = dst_g[sel][order] - lo
        deg = np.bincount(ed, minlength=NOWN)
        assert deg.max() <= CHE, f"degree {deg.max()} > {CHE}"
        # greedy chunks: <=WIN nodes, <=CHE edges, whole segments only
        chunks = []  # (node_lo, node_hi, edge_lo, edge_hi)
        nlo = 0
        eptr = 0
        cur_nodes = 0
        cur_edges = 0
        nhi = 0
        for n in range(NOWN):
            dg = deg[n]
            if cur_nodes + 1 > WIN or cur_edges + dg > CHE:
                chunks.append((nlo, nhi, eptr, eptr + cur_edges))
                eptr += cur_edges
                nlo = n
                cur_nodes = 0
                cur_edges = 0
            cur_nodes += 1
            cur_edges += dg
            nhi = n + 1
        chunks.append((nlo, nhi, eptr, eptr + cur_edges))
        cores.append(dict(es=es, ed=ed, chunks=chunks))
    R = max((len(c["chunks"]) + RPC - 1) // RPC for c in cores)
    R += (-R) % 4  # multiple of 4 (quarters, B-batches)
    C = R * RPC
    NPAD = R * 128
    gidx1 = np.zeros((NC, 128, C), np.int32)
    dstj = np.full((NC, 128, C), -1.0, np.float32)
    nodemap = np.full((NC, NOWN), -1, np.int64)
    for c in range(NC):
        es, ed, chunks = cores[c]["es"], cores[c]["ed"], cores[c]["chunks"]
        for k, (nlo, nhi, elo, ehi) in enumerate(chunks):
            ne = ehi - elo
            assert ne <= CHE and nhi - nlo <= WIN
            gidx1[c, :ne, k] = es[elo:ehi]
            dstj[c, :ne, k] = (ed[elo:ehi] - nlo).astype(np.float32)
            reg, kk = k // RPC, k % RPC
            rows = 128 * reg + WIN * kk + np.arange(nhi - nlo)
            nodemap[c, nlo:nhi] = rows
    # layer-1 table: interleaved row order (row of node n = (n%128)*TN1 + n//128)
    TN1 = (((N + 127) // 128 + 15) // 16) * 16
    # layer-2 table: quarter-major [q][owner][QR rows]; QR = NPAD//4
    QR = NPAD // 4
    gidx2 = np.zeros((NC, 128, C), np.int32)
    for c in range(NC):
        g = gidx1[c].astype(np.int64)
        own = g // NOWN
        loc = g % NOWN
        pr = nodemap[own, loc]
        q = pr // QR
        gidx2[c] = (q * (NC * QR) + own * QR + (pr % QR)).astype(np.int32)
    gidx1v = ((gidx1.astype(np.int64) % 128) * TN1 + gidx1.astype(np.int64) // 128).astype(np.int32)
    # static one-hot (dstj == j), bf16, laid out [128, C*8]
    oh = (dstj[:, :, :, None] == np.arange(8, dtype=np.float32)[None, None, None, :])
    onehot = oh.astype(BF16).reshape(NC, 128, C * 8)
    return cores, R, C, NPAD, gidx1v, gidx2, onehot, nodemap


def _build(prep):
    import os as _os
    CUT = _os.environ.get("K_CUT", "all")
    import concourse.bass as bass
    import concourse.bacc as bacc
    import concourse.mybir as mybir
    import concourse.tile as tile

    cores, R, C, NPAD, gidx1, gidx2, onehot, nodemap = prep
    f32, i32, bf16 = mybir.dt.float32, mybir.dt.int32, mybir.dt.bfloat16
    TN1 = (((N + 127) // 128 + 15) // 16) * 16  # node tiles, mult of 16
    NF1 = TN1 * 128
    QR = NPAD // 4          # rows per quarter (own block)
    RQ = R // 4             # regions per quarter

    nc = bacc.Bacc("TRN2", target_bir_lowering=False, debug=False, num_devices=NC)
    t_xTf = nc.dram_tensor("xTf", [128, NF1], bf16, kind="ExternalInput")
    t_xpT = nc.dram_tensor("xpT", [128, NPAD], bf16, kind="ExternalInput")
    t_g1 = nc.dram_tensor("g1", [128, C], i32, kind="ExternalInput")
    t_g2 = nc.dram_tensor("g2", [128, C], i32, kind="ExternalInput")
    t_oh = nc.dram_tensor("oh", [128, C * 8], bf16, kind="ExternalInput")
    t_we1 = nc.dram_tensor("we1", [128, 65], bf16, kind="ExternalInput")
    t_wl1 = nc.dram_tensor("wl1", [128, 64], bf16, kind="ExternalInput")
    t_wd1 = nc.dram_tensor("wd1", [128, 1], bf16, kind="ExternalInput")
    t_b1 = nc.dram_tensor("b1", [1, 64], bf16, kind="ExternalInput")
    t_we2 = nc.dram_tensor("we2", [64, 65], bf16, kind="ExternalInput")
    t_wl2 = nc.dram_tensor("wl2", [64, 64], bf16, kind="ExternalInput")
    t_wd2 = nc.dram_tensor("wd2", [64, 1], bf16, kind="ExternalInput")
    t_b2 = nc.dram_tensor("b2", [1, 64], bf16, kind="ExternalInput")
    t_idf = nc.dram_tensor("idf", [128, 128], f32, kind="ExternalInput")
    t_idb = nc.dram_tensor("idb", [128, 128], bf16, kind="ExternalInput")
    t_on = nc.dram_tensor("ones1", [1, 128], bf16, kind="ExternalInput")
    t_out = nc.dram_tensor("outp", [128, R * 64], f32, kind="ExternalOutput")
    # build-nonce: source-hash-dependent shape busts any stale executable
    # cache keyed on the HLO signature (the bass module is referenced by
    # handle, not content, in the custom call).
    import hashlib as _hl
    _nl = (int(_hl.sha1(open(__file__, "rb").read()).hexdigest(), 16) % 61) + 3
    t_nonce = nc.dram_tensor("nonce", [1, _nl], f32, kind="ExternalInput")
    nc._nonce_len = _nl
    DBG = _os.environ.get("K_DBG", "") == "1"
    if DBG:
        d_tb1 = nc.dram_tensor("d_tb1", [2048, 33], f32, kind="ExternalOutput")
        d_g = nc.dram_tensor("d_g", [128, 64 * 65], bf16, kind="ExternalOutput")
        d_adbs = nc.dram_tensor("d_adbs", [128, 512], bf16, kind="ExternalOutput")
        d_ind = nc.dram_tensor("d_ind", [128, 64 * 8], bf16, kind="ExternalOutput")
        d_acc = nc.dram_tensor("d_acc", [65, 128], f32, kind="ExternalOutput")
        d_h = nc.dram_tensor("d_h", [128, 64], bf16, kind="ExternalOutput")
        d_xlad1 = nc.dram_tensor("d_xlad1", [128, R * 64], bf16, kind="ExternalOutput")
        d_adf1 = nc.dram_tensor("d_adf1", [1, NPAD], bf16, kind="ExternalOutput")
        d_t2own = nc.dram_tensor("d_t2own", [512, 33], f32, kind="ExternalOutput")
        d_tb2 = nc.dram_tensor("d_tb2", [2048, 33], f32, kind="ExternalOutput")
        d_g2 = nc.dram_tensor("d_g2", [128, 64 * 65], bf16, kind="ExternalOutput")
        d_acc2 = nc.dram_tensor("d_acc2", [65, 128], f32, kind="ExternalOutput")

    EXP = mybir.ActivationFunctionType.Exp
    RELU = mybir.ActivationFunctionType.Relu

    def cp(use_vec, out, in_):
        if use_vec:
            nc.vector.tensor_copy(out, in_)
        else:
            nc.scalar.copy(out, in_)
    MULT = mybir.AluOpType.mult
    ADD = mybir.AluOpType.add
    MAX = mybir.AluOpType.max

    with tile.TileContext(nc) as tc:
        with (
            tc.tile_pool(name="dram", bufs=1, space="DRAM") as dpool,
            tc.tile_pool(name="const", bufs=1) as cpool,
        ):
            tb1 = dpool.tile([NF1, 33], f32)
            t2own = dpool.tile([NPAD, 33], f32)
            tb2 = dpool.tile([NC * NPAD, 33], f32)
            adfd1 = dpool.tile([R, 128], bf16)
            adfd2 = dpool.tile([R, 128], bf16)

            nonce = cpool.tile([1, t_nonce.shape[1]], f32, name="nonce")
            nc.sync.dma_start(nonce[:], t_nonce[:])
            we1 = cpool.tile([128, 65], bf16); nc.sync.dma_start(we1[:], t_we1[:])
            wl1 = cpool.tile([128, 64], bf16); nc.sync.dma_start(wl1[:], t_wl1[:])
            wd1 = cpool.tile([128, 1], bf16); nc.sync.dma_start(wd1[:], t_wd1[:])
            b1 = cpool.tile([1, 64], bf16); nc.sync.dma_start(b1[:], t_b1[:])
            we2 = cpool.tile([64, 65], bf16); nc.sync.dma_start(we2[:], t_we2[:])
            wl2 = cpool.tile([64, 64], bf16); nc.sync.dma_start(wl2[:], t_wl2[:])
            wd2 = cpool.tile([64, 1], bf16); nc.sync.dma_start(wd2[:], t_wd2[:])
            b2 = cpool.tile([1, 64], bf16); nc.sync.dma_start(b2[:], t_b2[:])
            idf = cpool.tile([128, 128], f32); nc.sync.dma_start(idf[:], t_idf[:])
            idb = cpool.tile([128, 128], bf16); nc.sync.dma_start(idb[:], t_idb[:])
            on1 = cpool.tile([1, 128], bf16); nc.sync.dma_start(on1[:], t_on[:])
            g1sb = cpool.tile([128, C], i32); nc.sync.dma_start(g1sb[:], t_g1[:])
            g2sb = cpool.tile([128, C], i32); nc.sync.dma_start(g2sb[:], t_g2[:])
            ohsb = cpool.tile([128, C, 8], bf16)
            nc.sync.dma_start(ohsb[:], t_oh[:].rearrange("p (c j) -> p c j", j=8))
            xlad1 = cpool.tile([128, R, 64], bf16)
            t2q = cpool.tile([128, 2, R // 4, 65], bf16)
            xlad2 = cpool.tile([128, R, 64], f32)
            ad1c = cpool.tile([128, R], bf16)
            ad2c = cpool.tile([128, R], bf16)
            adf1 = cpool.tile([1, NPAD], bf16)
            adf2 = cpool.tile([1, NPAD], bf16)

            # ---- phase A: table1 = x_full @ We1 (bf16 rows [xs|a_s]) ----
            with (
                tc.tile_pool(name="pa", bufs=3) as pa,
                tc.tile_pool(name="pap", bufs=2, space="PSUM") as pap,
            ):
                for g in range(TN1 // 16 if CUT != "a2" else 0):
                    xt = pa.tile([128, 2048], bf16, tag="xt")
                    nc.sync.dma_start(xt[:], t_xTf[:, 2048 * g : 2048 * (g + 1)])
                    ot = pa.tile([128, 16, 65], bf16, tag="ot")
                    for h in range(4):
                        ps = pap.tile([128, 260], f32, tag=f"ps{h % 2}")
                        for i in range(4):
                            t = 4 * h + i
                            nc.tensor.matmul(
                                out=ps[:, 65 * i : 65 * (i + 1)],
                                lhsT=xt[:, 128 * t : 128 * (t + 1)], rhs=we1[:],
                                start=True, stop=True,
                            )
                        cp(h % 2 == 0,
                           ot[:, 4 * h : 4 * (h + 1), :].rearrange("p a e -> p (a e)"),
                           ps[:])
                    nc.sync.dma_start(
                        tb1[:].bitcast(bf16).rearrange("(p t) e -> p t e", p=128)[
                            :, 16 * g : 16 * g + 16, 0:65
                        ],
                        ot[:],
                    )

            # ---- phase A2: xl/a_d for layer 1 from x_pad ----
            with (
                tc.tile_pool(name="pb", bufs=3) as pb,
                tc.tile_pool(name="pbp", bufs=2, space="PSUM") as pbp,
            ):
                for rb in range(R // B):
                    xt = pb.tile([128, B * 128], bf16, tag="xt2")
                    nc.sync.dma_start(xt[:], t_xpT[:, B * 128 * rb : B * 128 * (rb + 1)])
                    for q in range(B):
                        r = B * rb + q
                        lh = xt[:, 128 * q : 128 * (q + 1)]
                        ps = pbp.tile([128, 64], f32, tag="psl")
                        nc.tensor.matmul(out=ps[:], lhsT=lh, rhs=wl1[:], start=True, stop=True)
                        nc.tensor.matmul(out=ps[:], lhsT=on1[:], rhs=b1[:], start=False, stop=True)
                        cp(q % 2 == 0, xlad1[:, r, :], ps[:])
                        psd = pbp.tile([128, 1], f32, tag="psd")
                        nc.tensor.matmul(out=psd[:], lhsT=lh, rhs=wd1[:], start=True, stop=True)
                        nc.vector.tensor_copy(ad1c[:, r : r + 1], psd[:])
                # a_d row form [1, NPAD] via transpose + sbuf->sbuf DMA
                pst = pbp.tile([R, 128], bf16, tag="pst")
                nc.tensor.transpose(out=pst[:], in_=ad1c[:], identity=idb[:])
                adT1 = pb.tile([R, 128], bf16, tag="adT1")
                nc.scalar.copy(adT1[:], pst[:])
                nc.sync.dma_start(adfd1[:], adT1[:])
                nc.sync.dma_start(
                    adf1[:], adfd1[:].rearrange("r p -> (r p)").unsqueeze(0)
                )

            # ---- gather/softmax/scatter layer ----
            def gat_layer(table, gsb, adf, finish, tail, lyr=1):
                with (
                    tc.tile_pool(name="pg", bufs=2) as pg,
                    tc.tile_pool(name="pgs", bufs=2) as pgs,
                    tc.tile_pool(name="pgp", bufs=2, space="PSUM") as pgp,
                    tc.tile_pool(name="pgq", bufs=1, space="PSUM") as pgq,
                ):
                    NCH = B * RPC  # chunks per batch
                    for rb in range(R // B):
                        Gf = pg.tile([128, NCH, 33], f32, tag="G", name="Gf")
                        G = Gf[:].bitcast(bf16)  # [128, NCH, 66]
                        for kk in range(NCH):
                            nc.gpsimd.indirect_dma_start(
                                out=Gf[:, kk, :], out_offset=None, in_=table[:],
                                in_offset=bass.IndirectOffsetOnAxis(
                                    ap=gsb[:, NCH * rb + kk : NCH * rb + kk + 1],
                                    axis=0,
                                ),
                            )
                        adB = pgq.tile([128, B * 128], f32, tag="adB")
                        nc.tensor.matmul(
                            out=adB[:], lhsT=on1[:],
                            rhs=adf[:, B * 128 * rb : B * 128 * (rb + 1)],
                            start=True, stop=True,
                        )
                        adBs = pgs.tile([128, B * 128], bf16, tag="adBs")
                        nc.scalar.copy(adBs[:], adB[:])
                        grid = pgs.tile([128, NCH, 8], bf16, tag="grid")
                        nc.vector.tensor_tensor(
                            out=grid[:],
                            in0=G[:, :, 64:65].to_broadcast([128, NCH, 8]),
                            in1=adBs[:].rearrange("p (a b) -> p a b", b=8),
                            op=ADD,
                        )
                        grid2 = pgs.tile([128, NCH, 8], bf16, tag="grid2")
                        nc.vector.tensor_scalar(grid2[:], grid[:], NEG, None, op0=MULT)
                        nc.vector.tensor_tensor(out=grid[:], in0=grid[:], in1=grid2[:], op=MAX)
                        nc.scalar.activation(grid[:], grid[:], EXP)
                        ind = pgs.tile([128, NCH, 8], bf16, tag="ind")
                        nc.vector.tensor_tensor(
                            out=ind[:], in0=grid[:],
                            in1=ohsb[:, NCH * rb : NCH * (rb + 1), :], op=MULT,
                        )
                        if DBG and rb == 0 and lyr == 1:
                            nc.sync.dma_start(d_adbs[:], adBs[:])
                            nc.sync.dma_start(d_ind[:], ind[:].rearrange("p a e -> p (a e)"))
                        if DBG and rb == 0 and lyr == 2:
                            pass  # dbg tap disabled (bitcast view)
                        nc.vector.memset(G[:, :, 64:65], 1.0)
                        for q in range(B):
                            r = B * rb + q
                            acc = pgp.tile([65, 128], f32, tag="acc", name="acc", bufs=2)
                            for k in range(RPC):
                                nc.tensor.matmul(
                                    out=acc[:, 8 * k : 8 * (k + 1)],
                                    lhsT=G[:, RPC * q + k, 0:65], rhs=ind[:, RPC * q + k, :],
                                    start=True, stop=True,
                                )
                            accs = pg.tile([65, 128], f32, tag="accs")
                            cp(q % 2 == 0, accs[:], acc[:])
                            if DBG and r == 0 and lyr == 1:
                                nc.sync.dma_start(d_acc[:], accs[:])
                            if DBG and r == 0 and lyr == 2:
                                nc.sync.dma_start(d_acc2[:], accs[:])
                            accT = pgp.tile([128, 65], f32, tag="accT", name="accT", bufs=1)
                            nc.tensor.transpose(out=accT[:], in_=accs[:], identity=idf[0:65, 0:65])
                            den = pg.tile([128, 1], f32, tag="den")
                            nc.vector.tensor_scalar(den[:], accT[:, 64:65], 1e-16, None, op0=ADD)
                            rcp = pg.tile([128, 1], f32, tag="rcp")
                            nc.vector.reciprocal(rcp[:], den[:])
                            finish(rb, q, r, accT, rcp, pg, pgp)
                        tail(rb)

            # layer 1 finish: h -> project table2 rows + xl2/ad2
            t2s4 = [None]
            ob4 = [None]

            def finish1(rb, q, r, accT, rcp, pg, pgp):
                hsb = pg.tile([128, 64], bf16, tag="hsb")
                nc.vector.tensor_scalar(hsb[:], accT[:, 0:64], rcp[:], None, op0=MULT)
                nc.vector.tensor_tensor(out=hsb[:], in0=hsb[:], in1=xlad1[:, r, :], op=ADD)
                nc.scalar.activation(hsb[:], hsb[:], RELU)
                if DBG and r == 0:
                    nc.sync.dma_start(d_h[:], hsb[:])
                psT = pgp.tile([64, 128], bf16, tag="psT", name="psT", bufs=1)
                nc.tensor.transpose(out=psT[:], in_=hsb[:], identity=idb[:])
                hTs = pg.tile([64, 128], bf16, tag="hTs")
                cp(q % 2 == 1, hTs[:], psT[:])
                pfin = pgp.tile([128, 130], f32, tag="pfin", name="pfin", bufs=1)
                nc.tensor.matmul(out=pfin[:, 0:65], lhsT=hTs[:], rhs=we2[:], start=True, stop=True)
                nc.tensor.matmul(out=pfin[:, 65:129], lhsT=hTs[:], rhs=wl2[:], start=True, stop=False)
                nc.tensor.matmul(out=pfin[:, 65:129], lhsT=on1[:], rhs=b2[:], start=False, stop=True)
                nc.tensor.matmul(out=pfin[:, 129:130], lhsT=hTs[:], rhs=wd2[:], start=True, stop=True)
                qq, i = r // RQ, r % RQ
                nc.scalar.copy(t2q[:, qq % 2, i, :], pfin[:, 0:65])
                nc.vector.tensor_copy(xlad2[:, r, :], pfin[:, 65:129])
                nc.vector.tensor_copy(ad2c[:, r : r + 1], pfin[:, 129:130])
                if i == RQ - 1:
                    nc.sync.dma_start(
                        t2own[:].bitcast(bf16)[
                            QR * qq : QR * (qq + 1), 0:65
                        ].rearrange("(a p) e -> p a e", p=128),
                        t2q[:, qq % 2, :, :],
                    )
                    nc.gpsimd.collective_compute(
                        "AllGather", mybir.AluOpType.bypass,
                        replica_groups=[list(range(NC))],
                        ins=[t2own[QR * qq : QR * (qq + 1), :].opt()],
                        outs=[tb2[NC * QR * qq : NC * QR * (qq + 1), :].opt()],
                    )

            if CUT not in ("a", "a2"):
                gat_layer(tb1, g1sb, adf1, finish1, lambda rb: None)

            # a_d row form for layer 2
            if CUT == "all":
                with (
                    tc.tile_pool(name="pc", bufs=1) as pc,
                    tc.tile_pool(name="pcp", bufs=1, space="PSUM") as pcp,
                ):
                    pst2 = pcp.tile([R, 128], bf16, tag="pst2")
                    nc.tensor.transpose(out=pst2[:], in_=ad2c[:], identity=idb[:])
                    adT2 = pc.tile([R, 128], bf16, tag="adT2")
                    nc.scalar.copy(adT2[:], pst2[:])
                    nc.sync.dma_start(adfd2[:], adT2[:])
                    nc.sync.dma_start(
                        adf2[:], adfd2[:].rearrange("r p -> (r p)").unsqueeze(0)
                    )

            # layer 2 finish: out rows (batched DMA per B regions)
            def finish2(rb, q, r, accT, rcp, pg, pgp):
                if q == 0:
                    ob4[0] = pg.tile([128, B, 64], f32, tag="ob4", name="ob4")
                hsb = pg.tile([128, 64], f32, tag="hsb2")
                nc.vector.tensor_scalar(hsb[:], accT[:, 0:64], rcp[:], None, op0=MULT)
                nc.vector.tensor_tensor(out=ob4[0][:, q, :], in0=hsb[:], in1=xlad2[:, r, :], op=ADD)
                if q == B - 1:
                    nc.sync.dma_start(
                        t_out[:, 64 * B * rb : 64 * B * (rb + 1)].rearrange(
                            "p (a e) -> p a e", e=64
                        ),
                        ob4[0][:],
                    )

            if CUT == "all":
                gat_layer(tb2, g2sb, adf2, finish2, lambda rb: None, lyr=2)
            if DBG:
                nc.sync.dma_start(d_tb1[:], tb1[0:2048, :])
                nc.sync.dma_start(d_xlad1[:], xlad1[:].rearrange("p r e -> p (r e)"))
                nc.sync.dma_start(d_adf1[:], adf1[:])
                nc.sync.dma_start(d_t2own[:], t2own[0:512, :])
                nc.sync.dma_start(d_tb2[:], tb2[0:2048, :])

    nc.finalize()
    return nc


def _run(nc, inp, prep):
    from concourse.bass_utils import run_bass_kernel_spmd

    cores, R, C, NPAD, gidx1, gidx2, onehot, nodemap = prep
    in_maps = []
    for c in range(NC):
        in_maps.append({
            "xTf": inp["xTf"], "xpT": inp["xpT"][c], "g1": gidx1[c], "g2": gidx2[c],
            "oh": onehot[c],
            "we1": inp["we1"], "wl1": inp["wl1"], "wd1": inp["wd1"], "b1": inp["b1"],
            "we2": inp["we2"], "wl2": inp["wl2"], "wd2": inp["wd2"], "b2": inp["b2"],
            "idf": np.eye(128, dtype=np.float32),
            "nonce": np.zeros((1, nc._nonce_len), np.float32),
            "idb": np.eye(128, dtype=BF16),
            "ones1": np.ones((1, 128), BF16),
        })
    globals()["_LAST_NC"] = nc
    globals()["_LAST_INMAPS"] = in_maps
    res = run_bass_kernel_spmd(nc, in_maps, core_ids=list(range(NC)))
    globals()["_LAST_RESULTS"] = res.results
    return [r["outp"] for r in res.results]


def kernel(**inputs):
    x = np.asarray(inputs["x"], np.float32)
    ei = np.asarray(inputs["edge_index"])
    prep = _prep(ei)
    cores, R, C, NPAD, gidx1, gidx2, onehot, nodemap = prep

    W_src1 = np.asarray(inputs["W_src1"], np.float32)
    W_dst1 = np.asarray(inputs["W_dst1"], np.float32)
    att_src1 = np.asarray(inputs["att_src1"], np.float32)
    att_dst1 = np.asarray(inputs["att_dst1"], np.float32)
    bias1 = np.asarray(inputs["bias1"], np.float32)
    Wl1 = np.asarray(inputs["Wl1"], np.float32)
    bl1 = np.asarray(inputs["bl1"], np.float32)
    W_src2 = np.asarray(inputs["W_src2"], np.float32)
    W_dst2 = np.asarray(inputs["W_dst2"], np.float32)
    att_src2 = np.asarray(inputs["att_src2"], np.float32)
    att_dst2 = np.asarray(inputs["att_dst2"], np.float32)
    bias2 = np.asarray(inputs["bias2"], np.float32)
    Wl2 = np.asarray(inputs["Wl2"], np.float32)
    bl2 = np.asarray(inputs["bl2"], np.float32)

    TN1 = (((N + 127) // 128 + 15) // 16) * 16
    NF1 = TN1 * 128
    xf = np.zeros((NF1, D), np.float32)
    xf[:N] = x
    xTf = np.ascontiguousarray(xf.T).astype(BF16)  # [128, NF1]
    xpT = np.zeros((NC, D, NPAD), np.float32)
    for c in range(NC):
        xp = np.zeros((NPAD, D), np.float32)
        rows = nodemap[c]
        xp[rows] = x[c * NOWN : (c + 1) * NOWN]
        xpT[c] = xp.T
    inp = dict(
        xTf=xTf, xpT=xpT.astype(BF16),
        we1=np.concatenate([W_src1, (W_src1 @ att_src1)[:, None]], 1).astype(BF16),
        wl1=Wl1.astype(BF16), wd1=(W_dst1 @ att_dst1)[:, None].astype(BF16),
        b1=(bias1 + bl1)[None, :].astype(BF16),
        we2=np.concatenate([W_src2, (W_src2 @ att_src2)[:, None]], 1).astype(BF16),
        wl2=Wl2.astype(BF16), wd2=(W_dst2 @ att_dst2)[:, None].astype(BF16),
        b2=(bias2 + bl2)[None, :].astype(BF16),
    )
    nc = _build(prep)
    outs = _run(nc, inp, prep)
    full = np.zeros((N, H), np.float32)
    for c in range(NC):
        o = np.asarray(outs[c]).reshape(128, R, 64)
        prs = nodemap[c]
        full[c * NOWN : (c + 1) * NOWN] = o[prs % 128, prs // 128]
    return full



# revision 3
# speedup vs baseline: 1.3454x; 1.3454x over previous
"""Two-layer GAT on 8 TRN2 NeuronCores (Bass/Tile) — v3 (dma_gather).

Key change vs v2: edge-row gathers use InstDMAGatherAnt (2048 rows/op,
~5.4ns/row Q7 generation) instead of per-chunk indirect DMA (~1.2us per
128-row op). Structure:
- Own dst nodes bin-packed into R=112 regions x 4 windows of 32 slots so
  every (region, window, src-group) bucket fits ONE 128-edge chunk, for
  BOTH group labelings (L1: src//25088 -> 4 tables of <=25088 rows each
  addressable by int16; L2: src%4 -> 4 AG-slice tables of 28672 rows).
- Per 4-region batch: 4 dma_gather ops (one per group, 16 chunks = 2048
  rows each) into Gf [128, 64chunks, 128bf16]; 256B rows = [a_s|xs|pad].
- Weights: grid = a_s + a_d(window-bcast) -> leaky -> exp(ACT) -> mask by
  (dstw == iota32); PE per chunk: acc[65, 32w:+32] += G[:,kc,0:65]^T @
  ohw[:,kc,:], accumulating the 4 groups in PSUM.
- Layer-1 finish projects table2 rows (packed [a_s2|xs2] 66 bf16); per
  28-region slice q: AllGather (Shared out) then pitch-expand to 256B
  rows. L2 group q gathers depend only on AG slice q (overlap).
"""
import numpy as np
import ml_dtypes

BF16 = ml_dtypes.bfloat16

N = 100000
E = 1600000
D = 128
H = 64
NC = 8
NOWN = N // NC
NEG = 0.2
NF1 = 100352          # 4 * 25088, >= N
G1R = 25088           # L1 group rows
R = 112               # regions (multiple of 4)
RQ = R // 4           # regions per q-slice
NPAD = R * 128        # 14336 padded own rows
QR = NPAD // 4        # 3584 own rows per q-slice
G2R = NC * QR         # 28672 L2 group rows
C = R * 16            # chunks per layer
B = 4                 # regions per batch
NB = R // B           # batches
NCH = B * 16          # chunks per batch
OPS = NB * 4          # dma_gather ops per layer
SOP = 128             # idx i16 slots per op per partition (2048/16)


def _pack_core(degs, node_glob):
    """FFD vector bin-packing for one core. degs [NOWN, 8]; returns
    rowmap [NOWN] padded row ids."""
    rowmap = np.full(NOWN, -1, np.int64)
    node_q = node_glob % 4
    for q in range(4):
        nodes = np.where(node_q == q)[0]
        nb = RQ * 4
        bin_load = np.zeros((nb, 8), np.int32)
        bin_n = np.zeros(nb, np.int32)
        order = nodes[np.argsort(-degs[nodes].max(1), kind="stable")]
        for n in order:
            d = degs[n]
            fits = (bin_n < 32) & np.all(bin_load + d <= 128, axis=1)
            idx = np.where(fits)[0]
            assert len(idx) > 0, "bin packing infeasible; raise R"
            b = int(idx[np.argmax(bin_load[idx].max(1))])
            r = q * RQ + b // 4
            w = b % 4
            rowmap[n] = r * 128 + w * 32 + bin_n[b]
            bin_load[b] += d
            bin_n[b] += 1
    return rowmap


def _prep(edge_index):
    src_g = edge_index[0].astype(np.int64)
    dst_g = edge_index[1].astype(np.int64)
    g1 = src_g // G1R
    g2 = src_g % 4
    rowmaps = []
    for c in range(NC):
        lo = c * NOWN
        sel = (dst_g >= lo) & (dst_g < lo + NOWN)
        s1, s2, dl = g1[sel], g2[sel], dst_g[sel] - lo
        degs = np.zeros((NOWN, 8), np.int32)
        np.add.at(degs, (dl, s1), 1)
        np.add.at(degs, (dl, s2 + 4), 1)
        rowmaps.append(_pack_core(degs, np.arange(lo, lo + NOWN)))
    rowmaps = np.stack(rowmaps)          # [NC, NOWN]
    # row of src at its owner (for L2 table indices)
    own = src_g // NOWN
    row_at_owner = rowmaps[own, src_g % NOWN]

    gidx = np.zeros((2, NC, 128, C), np.int32)
    dstw = np.full((2, NC, 128, C), -1.0, np.float32)
    for c in range(NC):
        lo = c * NOWN
        sel = np.where((dst_g >= lo) & (dst_g < lo + NOWN))[0]
        es, ed = src_g[sel], dst_g[sel] - lo
        rows = rowmaps[c][ed]
        r = rows // 128
        w = (rows % 128) // 32
        wloc = (rows % 32).astype(np.float32)
        GT = G1R // 128
        i1 = es - g1[sel] * G1R
        rao = row_at_owner[sel]
        for lyr, (gg, base) in enumerate((
            (g1[sel], (i1 % 128) * GT + i1 // 128),
            (g2[sel], own[sel] * QR + (rao % 128) * RQ + (rao // 128 - RQ * g2[sel])),
        )):
            col = (r // 4) * 64 + gg * 16 + (r % 4) * 4 + w
            order = np.argsort(col, kind="stable")
            cs, is_, ws_ = col[order], base[order], wloc[order]
            cnt = np.bincount(cs, minlength=C)
            assert cnt.max() <= 128, f"bucket overflow {cnt.max()}"
            lane = np.arange(len(cs)) - np.concatenate(
                ([0], np.cumsum(cnt)))[cs]
            gidx[lyr, c, lane, cs] = is_
            dstw[lyr, c, lane, cs] = ws_
    # wrap idx for dma_gather: per op (batch rb, group g): 16 chunks
    # [cols rb*64+g*16 .. +16), flat j = c_local*128 + lane,
    # wrapped [16, 128] at [j%16, j//16], replicated to 128 partitions.
    gbuf = np.zeros((2, NC, 128, OPS * SOP), np.int16)
    for lyr in range(2):
        for c in range(NC):
            cols = gidx[lyr, c]                      # [128, C]
            # [OPS, 16 chunks, 128 lanes] -> flat [OPS, 2048]
            t = cols.T.reshape(NB, 4, 16, 128)       # [rb, g, ch, lane]
            flat = t.reshape(OPS, 2048)
            wrapped = flat.reshape(OPS, SOP, 16).transpose(0, 2, 1)
            w16 = wrapped.reshape(OPS * 16, SOP)     # per-op [16, 128]
            full = np.tile(
                w16.reshape(OPS, 16, SOP), (1, 8, 1)
            ).transpose(1, 0, 2).reshape(128, OPS * SOP)
            gbuf[lyr, c] = full.astype(np.int16)
    return rowmaps, gidx, dstw, gbuf


def _build(prep):
    import os as _os
    CUT = _os.environ.get("K_CUT", "all")
    import concourse.bass as bass
    import concourse.bacc as bacc
    import concourse.mybir as mybir
    import concourse.tile as tile

    f32, i16, bf16 = mybir.dt.float32, mybir.dt.int16, mybir.dt.bfloat16
    TN1 = NF1 // 128           # 784 node tiles
    GT = G1R // 128            # 196 tiles per L1 group

    nc = bacc.Bacc("TRN2", target_bir_lowering=False, debug=False, num_devices=NC)
    t_xTf = nc.dram_tensor("xTf", [128, NF1], bf16, kind="ExternalInput")
    t_xpT = nc.dram_tensor("xpT", [128, NPAD], bf16, kind="ExternalInput")
    t_g1 = nc.dram_tensor("g1", [128, OPS * SOP], i16, kind="ExternalInput")
    t_g2 = nc.dram_tensor("g2", [128, OPS * SOP], i16, kind="ExternalInput")
    t_dw1 = nc.dram_tensor("dw1", [128, C], bf16, kind="ExternalInput")
    t_dw2 = nc.dram_tensor("dw2", [128, C], bf16, kind="ExternalInput")
    t_iota = nc.dram_tensor("iota", [128, 64 * 32], bf16, kind="ExternalInput")
    t_we1 = nc.dram_tensor("we1", [128, 65], bf16, kind="ExternalInput")
    t_wl1 = nc.dram_tensor("wl1", [128, 64], bf16, kind="ExternalInput")
    t_wd1 = nc.dram_tensor("wd1", [128, 1], bf16, kind="ExternalInput")
    t_b1 = nc.dram_tensor("b1", [1, 64], bf16, kind="ExternalInput")
    t_we2 = nc.dram_tensor("we2", [64, 65], bf16, kind="ExternalInput")
    t_wl2 = nc.dram_tensor("wl2", [64, 64], bf16, kind="ExternalInput")
    t_wd2 = nc.dram_tensor("wd2", [64, 1], bf16, kind="ExternalInput")
    t_b2 = nc.dram_tensor("b2", [1, 64], bf16, kind="ExternalInput")
    t_idf = nc.dram_tensor("idf", [128, 128], f32, kind="ExternalInput")
    t_idb = nc.dram_tensor("idb", [128, 128], bf16, kind="ExternalInput")
    t_on = nc.dram_tensor("ones1", [1, 128], bf16, kind="ExternalInput")
    t_out = nc.dram_tensor("outp", [128, R * 64], f32, kind="ExternalOutput")
    import hashlib as _hl
    _nl = (int(_hl.sha1(open(__file__, "rb").read() + CUT.encode()).hexdigest(), 16) % 61) + 3
    t_nonce = nc.dram_tensor("nonce", [1, _nl], f32, kind="ExternalInput")
    nc._nonce_len = _nl
    DBG = _os.environ.get("K_DBG", "") == "1"
    if DBG:
        d_tb1 = nc.dram_tensor("d_tb1", [1024, 128], bf16, kind="ExternalOutput")
        d_G = nc.dram_tensor("d_G", [128, 64 * 128], bf16, kind="ExternalOutput")
        d_ohw = nc.dram_tensor("d_ohw", [128, 64 * 32], bf16, kind="ExternalOutput")
        d_acc = nc.dram_tensor("d_acc", [65, 128], f32, kind="ExternalOutput")
        d_h = nc.dram_tensor("d_h", [128, 64], bf16, kind="ExternalOutput")
        d_t2 = nc.dram_tensor("d_t2", [1024, 66], bf16, kind="ExternalOutput")
        d_tb2x = nc.dram_tensor("d_tb2x", [1024, 128], bf16, kind="ExternalOutput")
        d_acc2 = nc.dram_tensor("d_acc2", [65, 128], f32, kind="ExternalOutput")

    EXP = mybir.ActivationFunctionType.Exp
    RELU = mybir.ActivationFunctionType.Relu
    MULT = mybir.AluOpType.mult
    ADD = mybir.AluOpType.add
    MAX = mybir.AluOpType.max
    ISEQ = mybir.AluOpType.is_equal

    def cp(use_vec, out, in_):
        if use_vec:
            nc.vector.tensor_copy(out, in_)
        else:
            nc.scalar.copy(out, in_)

    with tile.TileContext(nc) as tc:
        with (
            tc.tile_pool(name="dram", bufs=1, space="DRAM") as dpool,
            tc.tile_pool(name="const", bufs=1) as cpool,
        ):
            tb1p = [dpool.tile([G1R, 128], bf16, name=f"tb1p{g}") for g in range(4)]
            t2own = [dpool.tile([QR, 33], f32, name=f"t2own{q}")[:] for q in range(4)]
            tb2q = [dpool.tile([G2R, 33], f32, name=f"tb2q{q}")[:] for q in range(4)]
            tb2x = [dpool.tile([G2R, 128], bf16, name=f"tb2x{q}") for q in range(4)]
            adfd = dpool.tile([R, 128], bf16)

            nonce = cpool.tile([1, t_nonce.shape[1]], f32, name="nonce")
            nc.sync.dma_start(nonce[:], t_nonce[:])
            we1 = cpool.tile([128, 65], bf16); nc.sync.dma_start(we1[:], t_we1[:])
            wl1 = cpool.tile([128, 64], bf16); nc.sync.dma_start(wl1[:], t_wl1[:])
            wd1 = cpool.tile([128, 1], bf16); nc.sync.dma_start(wd1[:], t_wd1[:])
            b1 = cpool.tile([1, 64], bf16); nc.sync.dma_start(b1[:], t_b1[:])
            we2 = cpool.tile([64, 65], bf16); nc.sync.dma_start(we2[:], t_we2[:])
            wl2 = cpool.tile([64, 64], bf16); nc.sync.dma_start(wl2[:], t_wl2[:])
            wd2 = cpool.tile([64, 1], bf16); nc.sync.dma_start(wd2[:], t_wd2[:])
            b2 = cpool.tile([1, 64], bf16); nc.sync.dma_start(b2[:], t_b2[:])
            idf = cpool.tile([128, 128], f32); nc.sync.dma_start(idf[:], t_idf[:])
            idb = cpool.tile([128, 128], bf16); nc.sync.dma_start(idb[:], t_idb[:])
            on1 = cpool.tile([1, 128], bf16); nc.sync.dma_start(on1[:], t_on[:])
            iota = cpool.tile([128, 64, 32], bf16)
            nc.sync.dma_start(iota[:], t_iota[:].rearrange("p (c j) -> p c j", j=32))
            gsb = cpool.tile([128, OPS * SOP], i16, name="gsb")
            dw1 = cpool.tile([128, C, 1], bf16)
            nc.sync.dma_start(dw1[:], t_dw1[:].rearrange("p (c o) -> p c o", o=1))
            dw2 = cpool.tile([128, C, 1], bf16)
            nc.sync.dma_start(dw2[:], t_dw2[:].rearrange("p (c o) -> p c o", o=1))
            xlad1 = cpool.tile([128, R, 64], bf16)
            xlad2 = cpool.tile([128, R, 64], bf16)
            ad1c = cpool.tile([128, R], bf16)
            ad2c = cpool.tile([128, R], bf16)
            adf = cpool.tile([1, NPAD], bf16)
            t2q = cpool.tile([128, 2, RQ, 66], bf16)
            reg2048 = nc.gpsimd.to_reg(2048)

            nc.sync.dma_start(gsb[:], t_g1[:])

            # ---- phase A: 4 group tables, rows [a_s | xs | junk] bf16 ----
            if CUT != "a2":
                with (
                    tc.tile_pool(name="pa", bufs=2) as pa,
                    tc.tile_pool(name="pap", bufs=2, space="PSUM") as pap,
                ):
                    for g in range(4):
                        nst = (GT + 15) // 16
                        for st in range(nst):
                            tl = min(16, GT - 16 * st)
                            base = g * G1R + st * 2048
                            xt = pa.tile([128, 16 * 128], bf16, tag="xt")
                            nc.sync.dma_start(
                                xt[:, : tl * 128], t_xTf[:, base : base + tl * 128])
                            ot = pa.tile([128, 16, 128], bf16, tag="ot")
                            for h in range((tl + 3) // 4):
                                hn = min(4, tl - 4 * h)
                                ps = pap.tile([128, 260], f32, tag=f"ps{h % 2}")
                                for i in range(hn):
                                    t = 4 * h + i
                                    nc.tensor.matmul(
                                        out=ps[:, 65 * i : 65 * (i + 1)],
                                        lhsT=xt[:, 128 * t : 128 * (t + 1)],
                                        rhs=we1[:], start=True, stop=True)
                                cp(h % 2 == 0,
                                   ot[:, 4 * h : 4 * h + hn, 0:65],
                                   ps[:, : 65 * hn].rearrange(
                                       "p (a e) -> p a e", e=65))
                            # table rows p-major interleaved: node i (in group)
                            # -> row (i%128)*GT + i//128
                            nc.sync.dma_start(
                                tb1p[g][:].rearrange("(p t) e -> p t e", p=128)[
                                    :, 16 * st : 16 * st + tl, :],
                                ot[:, :tl, :])

            # ---- phase A2: xl1 / a_d1 from xpT ----
            with (
                tc.tile_pool(name="pb", bufs=2) as pb,
                tc.tile_pool(name="pbp", bufs=2, space="PSUM") as pbp,
            ):
                for rb in range(NB):
                    xt = pb.tile([128, B * 128], bf16, tag="xt2")
                    nc.sync.dma_start(xt[:], t_xpT[:, B * 128 * rb : B * 128 * (rb + 1)])
                    for q in range(B):
                        r = B * rb + q
                        lh = xt[:, 128 * q : 128 * (q + 1)]
                        ps = pbp.tile([128, 64], f32, tag="psl")
                        nc.tensor.matmul(out=ps[:], lhsT=lh, rhs=wl1[:], start=True, stop=True)
                        nc.tensor.matmul(out=ps[:], lhsT=on1[:], rhs=b1[:], start=False, stop=True)
                        cp(q % 2 == 0, xlad1[:, r, :], ps[:])
                        psd = pbp.tile([128, 1], f32, tag="psd")
                        nc.tensor.matmul(out=psd[:], lhsT=lh, rhs=wd1[:], start=True, stop=True)
                        nc.vector.tensor_copy(ad1c[:, r : r + 1], psd[:])
                pst = pbp.tile([R, 128], bf16, tag="pst")
                nc.tensor.transpose(out=pst[:], in_=ad1c[:], identity=idb[:])
                adT = pb.tile([R, 128], bf16, tag="adT")
                nc.scalar.copy(adT[:], pst[:])
                nc.sync.dma_start(adfd[:], adT[:])
                nc.sync.dma_start(adf[:], adfd[:].rearrange("r p -> (r p)").unsqueeze(0))

            # ---- gather/softmax/scatter layer ----
            def gat_layer(tables, dw, finish, lyr):
                with (
                    tc.tile_pool(name="pg", bufs=2) as pg,
                    tc.tile_pool(name="pgs", bufs=2) as pgs,
                    tc.tile_pool(name="pgp", bufs=2, space="PSUM") as pgp,
                    tc.tile_pool(name="pgq", bufs=1, space="PSUM") as pgq,
                ):
                    for rb in range(NB):
                        Gf = pg.tile([128, NCH, 128], bf16, tag="G", name="Gf")
                        for g in range(4):
                            op = rb * 4 + g
                            nc.gpsimd.dma_gather(
                                out_ap=Gf[:, 16 * g : 16 * (g + 1), :],
                                in_ap=tables[g][:],
                                idxs_ap=gsb[:, op * SOP : (op + 1) * SOP],
                                num_idxs=2048,
                                num_idxs_reg=reg2048,
                                elem_size=128,
                                single_packet=False,
                            )
                        adB = pgq.tile([128, B * 128], f32, tag="adB")
                        nc.tensor.matmul(
                            out=adB[:], lhsT=on1[:],
                            rhs=adf[:, B * 128 * rb : B * 128 * (rb + 1)],
                            start=True, stop=True)
                        adBs = pgs.tile([128, NCH, 32], bf16, tag="adBs")
                        for g in range(4):
                            cp(g % 2 == 0,
                               adBs[:, 16 * g : 16 * (g + 1), :],
                               adB[:].rearrange("p (x j) -> p x j", j=32))
                        grid = pgs.tile([128, NCH, 32], bf16, tag="grid")
                        nc.vector.tensor_tensor(
                            out=grid[:],
                            in0=Gf[:, :, 0:1].to_broadcast([128, NCH, 32]),
                            in1=adBs[:],
                            op=ADD)
                        grid2 = pgs.tile([128, NCH, 32], bf16, tag="grid2")
                        nc.vector.tensor_scalar(grid2[:], grid[:], NEG, None, op0=MULT)
                        nc.vector.tensor_tensor(out=grid[:], in0=grid[:], in1=grid2[:], op=MAX)
                        nc.scalar.activation(grid[:], grid[:], EXP)
                        cmp = pgs.tile([128, NCH, 32], bf16, tag="cmp")
                        nc.vector.tensor_tensor(
                            out=cmp[:],
                            in0=dw[:, NCH * rb : NCH * (rb + 1), :]
                                .to_broadcast([128, NCH, 32]),
                            in1=iota[:], op=ISEQ)
                        nc.vector.tensor_tensor(out=grid[:], in0=grid[:], in1=cmp[:], op=MULT)
                        if DBG and rb == 0 and lyr == 1:
                            nc.sync.dma_start(d_G[:], Gf[:].rearrange("p a e -> p (a e)"))
                            nc.sync.dma_start(d_ohw[:], grid[:].rearrange("p a e -> p (a e)"))
                        nc.vector.memset(Gf[:, :, 0:1], 1.0)
                        for r4 in range(B):
                            r = B * rb + r4
                            acc = pgp.tile([65, 128], f32, tag="acc", name="acc", bufs=2)
                            for w in range(4):
                                for g in range(4):
                                    kc = g * 16 + r4 * 4 + w
                                    nc.tensor.matmul(
                                        out=acc[:, 32 * w : 32 * (w + 1)],
                                        lhsT=Gf[:, kc, 0:65],
                                        rhs=grid[:, kc, :],
                                        start=(g == 0), stop=(g == 3))
                            accs = pg.tile([65, 128], f32, tag="accs")
                            cp(r4 % 2 == 0, accs[:], acc[:])
                            if DBG and r == 0 and lyr == 1:
                                nc.sync.dma_start(d_acc[:], accs[:])
                            if DBG and r == 0 and lyr == 2:
                                nc.sync.dma_start(d_acc2[:], accs[:])
                            accT = pgp.tile([128, 65], f32, tag="accT", name="accT", bufs=1)
                            nc.tensor.transpose(out=accT[:], in_=accs[:], identity=idf[0:65, 0:65])
                            den = pg.tile([128, 1], f32, tag="den")
                            nc.vector.tensor_scalar(den[:], accT[:, 0:1], 1e-16, None, op0=ADD)
                            rcp = pg.tile([128, 1], f32, tag="rcp")
                            nc.vector.reciprocal(rcp[:], den[:])
                            finish(rb, r4, r, accT, rcp, pg, pgp)

            # layer-1 finish: h -> project packed table2 rows + xl2/ad2
            import os as _os3
            FIN = _os3.environ.get("K_FIN", "all")

            def finish1(rb, r4, r, accT, rcp, pg, pgp):
                hsb = pg.tile([128, 64], bf16, tag="hsb")
                nc.vector.tensor_scalar(hsb[:], accT[:, 1:65], rcp[:], None, op0=MULT)
                nc.vector.tensor_tensor(out=hsb[:], in0=hsb[:], in1=xlad1[:, r, :], op=ADD)
                nc.scalar.activation(hsb[:], hsb[:], RELU)
                if DBG and r == 0:
                    nc.sync.dma_start(d_h[:], hsb[:])
                if FIN == "h":
                    return
                psT = pgp.tile([64, 128], bf16, tag="psT", name="psT", bufs=1)
                nc.tensor.transpose(out=psT[:], in_=hsb[:], identity=idb[:])
                hTs = pg.tile([64, 128], bf16, tag="hTs")
                cp(r4 % 2 == 1, hTs[:], psT[:])
                pfin = pgp.tile([128, 130], f32, tag="pfin", name="pfin", bufs=1)
                nc.tensor.matmul(out=pfin[:, 0:65], lhsT=hTs[:], rhs=we2[:], start=True, stop=True)
                nc.tensor.matmul(out=pfin[:, 65:129], lhsT=hTs[:], rhs=wl2[:], start=True, stop=False)
                nc.tensor.matmul(out=pfin[:, 65:129], lhsT=on1[:], rhs=b2[:], start=False, stop=True)
                nc.tensor.matmul(out=pfin[:, 129:130], lhsT=hTs[:], rhs=wd2[:], start=True, stop=True)
                qq, i = r // RQ, r % RQ
                nc.scalar.copy(t2q[:, qq % 2, i, 0:65], pfin[:, 0:65])
                nc.vector.tensor_copy(xlad2[:, r, :], pfin[:, 65:129])
                nc.vector.tensor_copy(ad2c[:, r : r + 1], pfin[:, 129:130])
                if FIN == "proj":
                    return
                if i == RQ - 1:
                    # t2own local rows p-major: own row r -> (r%128)*RQ + (r//128 - RQ*q)
                    nc.sync.dma_start(
                        t2own[qq].bitcast(bf16).rearrange("(p a) e -> p a e", p=128),
                        t2q[:, qq % 2, :, :])
                    if FIN == "t2":
                        return
                    import os as _os2
                    if _os2.environ.get("K_NOAG", "") != "1":
                        nc.gpsimd.collective_compute(
                            "AllGather", mybir.AluOpType.bypass,
                            replica_groups=[list(range(NC))],
                            ins=[t2own[qq].opt()],
                            outs=[tb2q[qq].opt()],
                        )
                    # pitch-expand 66->128 bf16 via SBUF (contiguous loads:
                    # one 28-row run per partition -> 128 descriptors/DMA)
                    for sub in range(8):
                        rows = G2R // 8  # 3584
                        ra = rows // 128  # 28
                        pkst = pg.tile([128, ra * 66], bf16, tag="pkst")
                        nc.sync.dma_start(
                            pkst[:],
                            tb2q[qq].bitcast(bf16)[sub * rows : (sub + 1) * rows, :]
                                .rearrange("(p a) e -> p (a e)", p=128))
                        if FIN == "pk":
                            continue
                        ext = pg.tile([128, ra, 128], bf16, tag="ext")
                        cp(sub % 2 == 0, ext[:, :, 0:66],
                           pkst[:].rearrange("p (a e) -> p a e", e=66))
                        if FIN == "ext":
                            continue
                        nc.sync.dma_start(
                            tb2x[qq][sub * rows : (sub + 1) * rows, :]
                                .rearrange("(p a) e -> p a e", p=128),
                            ext[:])
                    if DBG and qq == 0:
                        nc.sync.dma_start(d_t2[:], tb2q[0].bitcast(bf16)[0:1024, :])
                        nc.sync.dma_start(d_tb2x[:], tb2x[0][0:1024, :])

            if CUT == "g1":
                def finish0(rb, r4, r, accT, rcp, pg, pgp):
                    if r == 0:
                        hs0 = pg.tile([128, 64], f32, tag="hs0")
                        nc.vector.tensor_copy(hs0[:], accT[:, 1:65])
                        nc.sync.dma_start(
                            t_out[:, 0:64].rearrange("p (a e) -> p a e", e=64),
                            hs0[:].rearrange("p (a e) -> p a e", e=64))
                gat_layer(tb1p, dw1, finish0, lyr=1)
            elif CUT not in ("a", "a2"):
                gat_layer(tb1p, dw1, finish1, lyr=1)

            if CUT == "all":
                # a_d2 row + swap gather indices to layer 2
                with (
                    tc.tile_pool(name="pc", bufs=1) as pc,
                    tc.tile_pool(name="pcp", bufs=1, space="PSUM") as pcp,
                ):
                    pst2 = pcp.tile([R, 128], bf16, tag="pst2")
                    nc.tensor.transpose(out=pst2[:], in_=ad2c[:], identity=idb[:])
                    adT2 = pc.tile([R, 128], bf16, tag="adT2")
                    nc.scalar.copy(adT2[:], pst2[:])
                    nc.sync.dma_start(adfd[:], adT2[:])
                    nc.sync.dma_start(adf[:], adfd[:].rearrange("r p -> (r p)").unsqueeze(0))
                nc.sync.dma_start(gsb[:], t_g2[:])

                ob4 = [None]

                def finish2(rb, r4, r, accT, rcp, pg, pgp):
                    if r4 == 0:
                        ob4[0] = pg.tile([128, B, 64], f32, tag="ob4", name="ob4")
                    hsb = pg.tile([128, 64], f32, tag="hsb2")
                    nc.vector.tensor_scalar(hsb[:], accT[:, 1:65], rcp[:], None, op0=MULT)
                    nc.vector.tensor_tensor(out=ob4[0][:, r4, :], in0=hsb[:], in1=xlad2[:, r, :], op=ADD)
                    if r4 == B - 1:
                        nc.sync.dma_start(
                            t_out[:, 64 * B * rb : 64 * B * (rb + 1)]
                                .rearrange("p (a e) -> p a e", e=64),
                            ob4[0][:])

                gat_layer(tb2x, dw2, finish2, lyr=2)
            if DBG:
                nc.sync.dma_start(d_tb1[:], tb1p[0][0:1024, :])

    nc.finalize()
    return nc


def _run(nc, inp):
    from concourse.bass_utils import run_bass_kernel_spmd

    in_maps = []
    for c in range(NC):
        m = dict(inp["shared"])
        m.update(inp["cores"][c])
        m["nonce"] = np.zeros((1, nc._nonce_len), np.float32)
        in_maps.append(m)
    globals()["_LAST_NC"] = nc
    globals()["_LAST_INMAPS"] = in_maps
    res = run_bass_kernel_spmd(nc, in_maps, core_ids=list(range(NC)))
    globals()["_LAST_RESULTS"] = res.results
    return [r["outp"] for r in res.results]


def kernel(**inputs):
    x = np.asarray(inputs["x"], np.float32)
    ei = np.asarray(inputs["edge_index"])
    rowmaps, gidx, dstw, gbuf = _prep(ei)

    W_src1 = np.asarray(inputs["W_src1"], np.float32)
    att_src1 = np.asarray(inputs["att_src1"], np.float32)
    W_dst1 = np.asarray(inputs["W_dst1"], np.float32)
    att_dst1 = np.asarray(inputs["att_dst1"], np.float32)
    bias1 = np.asarray(inputs["bias1"], np.float32)
    Wl1 = np.asarray(inputs["Wl1"], np.float32)
    bl1 = np.asarray(inputs["bl1"], np.float32)
    W_src2 = np.asarray(inputs["W_src2"], np.float32)
    att_src2 = np.asarray(inputs["att_src2"], np.float32)
    W_dst2 = np.asarray(inputs["W_dst2"], np.float32)
    att_dst2 = np.asarray(inputs["att_dst2"], np.float32)
    bias2 = np.asarray(inputs["bias2"], np.float32)
    Wl2 = np.asarray(inputs["Wl2"], np.float32)
    bl2 = np.asarray(inputs["bl2"], np.float32)

    xf = np.zeros((NF1, D), np.float32)
    xf[:N] = x
    xTf = np.ascontiguousarray(xf.T).astype(BF16)
    iota = np.tile(np.arange(32, dtype=np.float32), 64)[None, :].repeat(128, 0)
    shared = dict(
        xTf=xTf,
        iota=iota.astype(BF16),
        we1=np.concatenate([(W_src1 @ att_src1)[:, None], W_src1], 1).astype(BF16),
        wl1=Wl1.astype(BF16), wd1=(W_dst1 @ att_dst1)[:, None].astype(BF16),
        b1=(bias1 + bl1)[None, :].astype(BF16),
        we2=np.concatenate([(W_src2 @ att_src2)[:, None], W_src2], 1).astype(BF16),
        wl2=Wl2.astype(BF16), wd2=(W_dst2 @ att_dst2)[:, None].astype(BF16),
        b2=(bias2 + bl2)[None, :].astype(BF16),
        idf=np.eye(128, dtype=np.float32),
        idb=np.eye(128, dtype=BF16),
        ones1=np.ones((1, 128), BF16),
    )
    cores = []
    for c in range(NC):
        xp = np.zeros((NPAD, D), np.float32)
        xp[rowmaps[c]] = x[c * NOWN : (c + 1) * NOWN]
        cores.append(dict(
            xpT=np.ascontiguousarray(xp.T).astype(BF16),
            g1=gbuf[0, c], g2=gbuf[1, c],
            dw1=dstw[0, c].astype(BF16), dw2=dstw[1, c].astype(BF16),
        ))
    nc = _build((rowmaps,))
    outs = _run(nc, dict(shared=shared, cores=cores))
    full = np.zeros((N, H), np.float32)
    for c in range(NC):
        o = np.asarray(outs[c]).reshape(128, R, 64)
        rm = rowmaps[c]
        full[c * NOWN : (c + 1) * NOWN] = o[rm % 128, rm // 128]
    return full
